# revision 4
# baseline (speedup 1.0000x reference)
"""Trainium2 Bass kernel for the ODEFunc GNN message-passing module.

Math (B=2, N=512, H=128, O=32):
    q = z @ Wq.T + bq ;  k = s_t @ Wk.T + bk
    scores = (q @ k.T)/sqrt(H), diagonal masked to -inf
    attn = softmax_j(scores)
    rel  = tanh(zi_i + zj_j + b1) @ W2.T + b2           (zi = z@W1i.T, zj = z@W1j.T)
    agg  = sum_j attn[i,j] * rel[i,j]
    dz   = tanh(agg @ W3.T + b3) @ W4.T + b4

Key algebraic simplification used here: softmax rows sum to 1, so
    agg = (sum_j attn[i,j] * tanh(zi_i + zj_j + b1)) @ W2.T + b2
i.e. the W2 matmul moves after the j-aggregation and the [N,N,H] "rel"
tensor is never multiplied by W2 pairwise.

Sharding: the 1024 (b, i) pairs are split over 8 cores (batch-major, 128
i's per core). Each core computes, with h on partitions:
    per i: V_i[h, j] = tanh(yjT[h,j] + xiT[h,i])        (one ACT op, bias trick)
           arep[h, j] = attn[i, j] broadcast over h     (PE rank-1 matmul w/ ones)
           U[:, i]    = sum_j V_i * arep                (one fused DVE op)
then the small MLP epilogue on [128, 128] tiles.
"""

import numpy as np

B, N, H, O = 2, 512, 128, 32
NC = 8
CPB = NC // B  # cores per batch = 4
IPC = N // CPB  # i's per core = 128

_CACHE = {}

# Stash of the last BassKernelResults (exec_time_ns etc.) for test harnesses.
LAST_RESULTS = None


def _build():
    from contextlib import ExitStack

    import concourse.tile as tile
    from concourse import bacc, mybir

    f32 = mybir.dt.float32
    AF = mybir.ActivationFunctionType
    ALU = mybir.AluOpType

    nc = bacc.Bacc(trn_type="TRN2")

    ins = {}

    def din(name, shape):
        ins[name] = nc.dram_tensor(name, shape, f32, kind="ExternalInput")
        return ins[name]

    zT = din("zT", [H, N])
    zTi = din("zTi", [H, IPC])
    sT = din("sT", [O, N])
    mask = din("mask", [IPC, N])
    ones = din("ones", [65, IPC])
    WqTs = din("WqTs", [H, H])
    bqs = din("bqs", [H, 1])
    WkT = din("WkT", [O, H])
    bk = din("bk", [H, 1])
    W1iT = din("W1iT", [H, H])
    b1 = din("b1", [H, 1])
    W1jT = din("W1jT", [H, H])
    W2T = din("W2T", [H, H])
    b2 = din("b2", [H, 1])
    W3T = din("W3T", [H, H])
    b3 = din("b3", [H, 1])
    W4T = din("W4T", [H, H])
    b4 = din("b4", [H, 1])
    out = nc.dram_tensor("out", [H, IPC], f32, kind="ExternalOutput")

    with tile.TileContext(nc) as tc, ExitStack() as ctx:
        const = ctx.enter_context(tc.tile_pool(name="const", bufs=1))
        work = ctx.enter_context(tc.tile_pool(name="work", bufs=2))
        vpool = ctx.enter_context(tc.tile_pool(name="vpool", bufs=3))
        ps = ctx.enter_context(tc.tile_pool(name="ps", bufs=2, space="PSUM"))
        apool = ctx.enter_context(tc.tile_pool(name="apool", bufs=3, space="PSUM"))

        def load(drt, shape, tag):
            t = const.tile(shape, f32, tag=tag, name=tag + "_sb")
            nc.sync.dma_start(t[:], drt[:, :])
            return t

        zT_t = load(zT, [H, N], "zT")
        zTi_t = load(zTi, [H, IPC], "zTi")
        sT_t = load(sT, [O, N], "sT")
        mask_t = load(mask, [IPC, N], "mask")
        ones_t = load(ones, [65, IPC], "ones")
        WqTs_t = load(WqTs, [H, H], "WqTs")
        bqs_t = load(bqs, [H, 1], "bqs")
        WkT_t = load(WkT, [O, H], "WkT")
        bk_t = load(bk, [H, 1], "bk")
        W1iT_t = load(W1iT, [H, H], "W1iT")
        b1_t = load(b1, [H, 1], "b1")
        W1jT_t = load(W1jT, [H, H], "W1jT")
        W2T_t = load(W2T, [H, H], "W2T")
        b2_t = load(b2, [H, 1], "b2")
        W3T_t = load(W3T, [H, H], "W3T")
        b3_t = load(b3, [H, 1], "b3")
        W4T_t = load(W4T, [H, H], "W4T")
        b4_t = load(b4, [H, 1], "b4")

        # kT[h, j] = Wk @ s_t[b].T + bk
        kT_ps = ps.tile([H, N], f32, tag="mm", name="kT_ps")
        nc.tensor.matmul(kT_ps[:], WkT_t[:], sT_t[:], start=True, stop=True)
        kT_t = const.tile([H, N], f32, tag="kT", name="kT_sb")
        nc.scalar.activation(kT_t[:], kT_ps[:], AF.Identity, bias=bk_t[:, 0:1])

        # qsT[h, i] = (Wq/sqrt(H)) @ z_i.T + bq/sqrt(H)
        qs_ps = ps.tile([H, IPC], f32, tag="mm", name="qs_ps")
        nc.tensor.matmul(qs_ps[:], WqTs_t[:], zTi_t[:], start=True, stop=True)
        qsT_t = work.tile([H, IPC], f32, tag="qsT", name="qsT_sb")
        nc.scalar.activation(qsT_t[:], qs_ps[:], AF.Identity, bias=bqs_t[:, 0:1])

        # scores[i, j] = qsT^T @ kT  (+ diagonal mask)
        sc_ps = ps.tile([IPC, N], f32, tag="mm", name="sc_ps")
        nc.tensor.matmul(sc_ps[:], qsT_t[:], kT_t[:], start=True, stop=True)
        sc_t = work.tile([IPC, N], f32, tag="sc", name="sc_sb")
        nc.vector.tensor_add(sc_t[:], sc_ps[:], mask_t[:])

        # softmax over j (free dim)
        mx = work.tile([IPC, 1], f32, tag="mx", name="mx")
        nc.vector.tensor_reduce(mx[:], sc_t[:], mybir.AxisListType.X, ALU.max)
        nmx = work.tile([IPC, 1], f32, tag="nmx", name="nmx")
        nc.vector.tensor_scalar_mul(nmx[:], mx[:], -1.0)
        et = work.tile([IPC, N], f32, tag="et", name="et")
        ssum = work.tile([IPC, 1], f32, tag="ssum", name="ssum")
        nc.scalar.activation(
            et[:], sc_t[:], AF.Exp, bias=nmx[:, 0:1], scale=1.0, accum_out=ssum[:]
        )
        rs = work.tile([IPC, 1], f32, tag="rs", name="rs")
        nc.vector.reciprocal(rs[:], ssum[:])
        attn = work.tile([IPC, N], f32, tag="attn", name="attn_sb")
        nc.vector.tensor_scalar_mul(attn[:], et[:], rs[:, 0:1])

        # Matmul operands must start at partition 0/32/64, so repack attn rows
        # into 3 partition groups with rows along the free dim.
        GRP = (IPC + 2) // 3  # 43 rows per group
        attn_rows = const.tile([65, GRP * N], f32, tag="attn_rows", name="attn_rows")
        for g in range(3):
            r0 = g * GRP
            r1 = min(IPC, r0 + GRP)
            nc.sync.dma_start(
                attn_rows[32 * g : 32 * g + 1, 0 : (r1 - r0) * N],
                attn[r0:r1, :],
            )

        # xiT[h, i] = W1i @ z_i.T + b1 ; yjT[h, j] = W1j @ z.T
        xi_ps = ps.tile([H, IPC], f32, tag="mm", name="xi_ps")
        nc.tensor.matmul(xi_ps[:], W1iT_t[:], zTi_t[:], start=True, stop=True)
        xiT_t = const.tile([H, IPC], f32, tag="xiT", name="xiT_sb")
        nc.scalar.activation(xiT_t[:], xi_ps[:], AF.Identity, bias=b1_t[:, 0:1])
        yj_ps = ps.tile([H, N], f32, tag="mm", name="yj_ps")
        nc.tensor.matmul(yj_ps[:], W1jT_t[:], zT_t[:], start=True, stop=True)
        yjT_t = const.tile([H, N], f32, tag="yjT", name="yjT_sb")
        nc.scalar.activation(yjT_t[:], yj_ps[:], AF.Identity, bias=0.0)

        # main loop over this core's 128 i's
        U = const.tile([H, IPC], f32, tag="U", name="U_sb")
        scratch = const.tile([H, N], f32, tag="scratch", name="scratch_sb")
        for i in range(IPC):
            g, r = divmod(i, GRP)
            arep = apool.tile([H, N], f32, tag="arep", name="arep")
            nc.tensor.matmul(
                arep[:],
                ones_t[32 * g : 32 * g + 1, :],
                attn_rows[32 * g : 32 * g + 1, r * N : (r + 1) * N],
                start=True,
                stop=True,
            )
            v = vpool.tile([H, N], f32, tag="v", name="v")
            nc.scalar.activation(
                v[:], yjT_t[:], AF.Tanh, bias=xiT_t[:, i : i + 1], scale=1.0
            )
            nc.vector.scalar_tensor_tensor(
                scratch[:],
                v[:],
                1.0,
                arep[:],
                ALU.mult,
                ALU.mult,
                accum_out=U[:, i : i + 1],
            )

        # epilogue MLP: agg = W2@U + b2 ; t3 = tanh(W3@agg + b3) ; dz = W4@t3 + b4
        c2 = ps.tile([H, IPC], f32, tag="mm", name="c2_ps")
        nc.tensor.matmul(c2[:], W2T_t[:], U[:], start=True, stop=True)
        agg = work.tile([H, IPC], f32, tag="agg", name="agg_sb")
        nc.scalar.activation(agg[:], c2[:], AF.Identity, bias=b2_t[:, 0:1])
        c3 = ps.tile([H, IPC], f32, tag="mm", name="c3_ps")
        nc.tensor.matmul(c3[:], W3T_t[:], agg[:], start=True, stop=True)
        t3 = work.tile([H, IPC], f32, tag="t3", name="t3_sb")
        nc.scalar.activation(t3[:], c3[:], AF.Tanh, bias=b3_t[:, 0:1])
        c4 = ps.tile([H, IPC], f32, tag="mm", name="c4_ps")
        nc.tensor.matmul(c4[:], W4T_t[:], t3[:], start=True, stop=True)
        dzT = work.tile([H, IPC], f32, tag="dzT", name="dzT_sb")
        nc.scalar.activation(dzT[:], c4[:], AF.Identity, bias=b4_t[:, 0:1])
        nc.sync.dma_start(out[:, :], dzT[:])

    nc.finalize()
    return nc


def _get_nc():
    if "nc" not in _CACHE:
        _CACHE["nc"] = _build()
    return _CACHE["nc"]


def kernel(**inputs):
    global LAST_RESULTS
    from concourse.bass_utils import run_bass_kernel_spmd

    z = np.asarray(inputs["z"], dtype=np.float32)
    s_t = np.asarray(inputs["s_t"], dtype=np.float32)
    W1 = np.asarray(inputs["W1"], dtype=np.float32)
    b1 = np.asarray(inputs["b1"], dtype=np.float32)
    W2 = np.asarray(inputs["W2"], dtype=np.float32)
    b2 = np.asarray(inputs["b2"], dtype=np.float32)
    Wq = np.asarray(inputs["Wq"], dtype=np.float32)
    bq = np.asarray(inputs["bq"], dtype=np.float32)
    Wk = np.asarray(inputs["Wk"], dtype=np.float32)
    bk = np.asarray(inputs["bk"], dtype=np.float32)
    W3 = np.asarray(inputs["W3"], dtype=np.float32)
    b3 = np.asarray(inputs["b3"], dtype=np.float32)
    W4 = np.asarray(inputs["W4"], dtype=np.float32)
    b4 = np.asarray(inputs["b4"], dtype=np.float32)

    rt = np.float32(1.0 / np.sqrt(H))
    col = lambda v: np.ascontiguousarray(v.reshape(H, 1), dtype=np.float32)
    tr = lambda m: np.ascontiguousarray(m.T, dtype=np.float32)

    shared = dict(
        ones=np.ones((65, IPC), np.float32),
        WqTs=tr(Wq) * rt,
        bqs=col(bq) * rt,
        WkT=tr(Wk),
        bk=col(bk),
        W1iT=tr(W1[:, :H]),
        b1=col(b1),
        W1jT=tr(W1[:, H:]),
        W2T=tr(W2),
        b2=col(b2),
        W3T=tr(W3),
        b3=col(b3),
        W4T=tr(W4),
        b4=col(b4),
    )

    in_maps = []
    for c in range(NC):
        b, blk = divmod(c, CPB)
        i0 = blk * IPC
        m = np.zeros((IPC, N), np.float32)
        m[np.arange(IPC), i0 + np.arange(IPC)] = np.float32(-1e30)
        in_maps.append(
            dict(
                shared,
                zT=tr(z[b]),
                zTi=tr(z[b, i0 : i0 + IPC]),
                sT=tr(s_t[b]),
                mask=m,
            )
        )

    nc = _get_nc()
    res = run_bass_kernel_spmd(nc, in_maps, core_ids=list(range(NC)))
    LAST_RESULTS = res

    dz = np.empty((B, N, H), dtype=np.float32)
    for c in range(NC):
        b, blk = divmod(c, CPB)
        i0 = blk * IPC
        dz[b, i0 : i0 + IPC, :] = res.results[c]["out"].T
    return dz


# revision 5
# speedup vs baseline: 4.5683x; 4.5683x over previous
"""Trainium2 Bass kernel for the ODEFunc GNN message-passing module.

Math (B=2, N=512, H=128, O=32):
    q = z @ Wq.T + bq ;  k = s_t @ Wk.T + bk
    scores = (q @ k.T)/sqrt(H), diagonal masked to -inf
    attn = softmax_j(scores)
    rel  = tanh(zi_i + zj_j + b1) @ W2.T + b2           (zi = z@W1i.T, zj = z@W1j.T)
    agg  = sum_j attn[i,j] * rel[i,j]
    dz   = tanh(agg @ W3.T + b3) @ W4.T + b4

Key algebraic simplification used here: softmax rows sum to 1, so
    agg = (sum_j attn[i,j] * tanh(zi_i + zj_j + b1)) @ W2.T + b2
i.e. the W2 matmul moves after the j-aggregation and the [N,N,H] "rel"
tensor is never multiplied by W2 pairwise.

Sharding: the 1024 (b, i) pairs are split over 8 cores (batch-major, 128
i's per core). Each core computes, with h on partitions:
    per i: V_i[h, j] = tanh(yjT[h,j] + xiT[h,i])        (one ACT op, bias trick)
           arep[h, j] = attn[i, j] broadcast over h     (PE rank-1 matmul w/ ones)
           U[:, i]    = sum_j V_i * arep                (one fused DVE op)
then the small MLP epilogue on [128, 128] tiles.
"""

import ml_dtypes
import numpy as np

B, N, H, O = 2, 512, 128, 32
NC = 8
CPB = NC // B  # cores per batch = 4
IPC = N // CPB  # i's per core = 128

_CACHE = {}

# Stash of the last BassKernelResults (exec_time_ns etc.) for test harnesses.
LAST_RESULTS = None


def _build():
    from contextlib import ExitStack

    import concourse.tile as tile
    from concourse import bacc, mybir

    f32 = mybir.dt.float32
    bf16 = mybir.dt.bfloat16
    AF = mybir.ActivationFunctionType
    ALU = mybir.AluOpType

    nc = bacc.Bacc(trn_type="TRN2")

    ins = {}

    def din(name, shape):
        ins[name] = nc.dram_tensor(name, shape, f32, kind="ExternalInput")
        return ins[name]

    zT = din("zT", [H, N])
    zTi = din("zTi", [H, IPC])
    sT = din("sT", [O, N])
    mask = din("mask", [IPC, N])
    ones = nc.dram_tensor("ones", [65, IPC], bf16, kind="ExternalInput")
    ins["ones"] = ones
    WqTs = din("WqTs", [H, H])
    bqs = din("bqs", [H, 1])
    WkT = din("WkT", [O, H])
    bk = din("bk", [H, 1])
    W1iT = din("W1iT", [H, H])
    b1 = din("b1", [H, 1])
    W1jT = din("W1jT", [H, H])
    W2T = din("W2T", [H, H])
    b2 = din("b2", [H, 1])
    W3T = din("W3T", [H, H])
    b3 = din("b3", [H, 1])
    W4T = din("W4T", [H, H])
    b4 = din("b4", [H, 1])
    out = nc.dram_tensor("out", [H, IPC], f32, kind="ExternalOutput")

    with tile.TileContext(nc) as tc, ExitStack() as ctx:
        const = ctx.enter_context(tc.tile_pool(name="const", bufs=1))
        work = ctx.enter_context(tc.tile_pool(name="work", bufs=2))
        vpool = ctx.enter_context(tc.tile_pool(name="vpool", bufs=3))
        ps = ctx.enter_context(tc.tile_pool(name="ps", bufs=2, space="PSUM"))
        apool = ctx.enter_context(tc.tile_pool(name="apool", bufs=3, space="PSUM"))

        def load(drt, shape, tag):
            t = const.tile(shape, f32, tag=tag, name=tag + "_sb")
            nc.sync.dma_start(t[:], drt[:, :])
            return t

        zT_t = load(zT, [H, N], "zT")
        zTi_t = load(zTi, [H, IPC], "zTi")
        sT_t = load(sT, [O, N], "sT")
        mask_t = load(mask, [IPC, N], "mask")
        ones_t = const.tile([65, IPC], bf16, tag="ones", name="ones_sb")
        nc.sync.dma_start(ones_t[:], ones[:, :])
        WqTs_t = load(WqTs, [H, H], "WqTs")
        bqs_t = load(bqs, [H, 1], "bqs")
        WkT_t = load(WkT, [O, H], "WkT")
        bk_t = load(bk, [H, 1], "bk")
        W1iT_t = load(W1iT, [H, H], "W1iT")
        b1_t = load(b1, [H, 1], "b1")
        W1jT_t = load(W1jT, [H, H], "W1jT")
        W2T_t = load(W2T, [H, H], "W2T")
        b2_t = load(b2, [H, 1], "b2")
        W3T_t = load(W3T, [H, H], "W3T")
        b3_t = load(b3, [H, 1], "b3")
        W4T_t = load(W4T, [H, H], "W4T")
        b4_t = load(b4, [H, 1], "b4")

        # kT[h, j] = Wk @ s_t[b].T + bk
        kT_ps = ps.tile([H, N], f32, tag="mm", name="kT_ps")
        nc.tensor.matmul(kT_ps[:], WkT_t[:], sT_t[:], start=True, stop=True)
        kT_t = const.tile([H, N], f32, tag="kT", name="kT_sb")
        nc.scalar.activation(kT_t[:], kT_ps[:], AF.Identity, bias=bk_t[:, 0:1])

        # qsT[h, i] = (Wq/sqrt(H)) @ z_i.T + bq/sqrt(H)
        qs_ps = ps.tile([H, IPC], f32, tag="mm", name="qs_ps")
        nc.tensor.matmul(qs_ps[:], WqTs_t[:], zTi_t[:], start=True, stop=True)
        qsT_t = work.tile([H, IPC], f32, tag="qsT", name="qsT_sb")
        nc.scalar.activation(qsT_t[:], qs_ps[:], AF.Identity, bias=bqs_t[:, 0:1])

        # scores[i, j] = qsT^T @ kT  (+ diagonal mask)
        sc_ps = ps.tile([IPC, N], f32, tag="mm", name="sc_ps")
        nc.tensor.matmul(sc_ps[:], qsT_t[:], kT_t[:], start=True, stop=True)
        sc_t = work.tile([IPC, N], f32, tag="sc", name="sc_sb")
        nc.vector.tensor_add(sc_t[:], sc_ps[:], mask_t[:])

        # softmax over j (free dim)
        mx = work.tile([IPC, 1], f32, tag="mx", name="mx")
        nc.vector.tensor_reduce(mx[:], sc_t[:], mybir.AxisListType.X, ALU.max)
        nmx = work.tile([IPC, 1], f32, tag="nmx", name="nmx")
        nc.vector.tensor_scalar_mul(nmx[:], mx[:], -1.0)
        et = work.tile([IPC, N], f32, tag="et", name="et")
        ssum = work.tile([IPC, 1], f32, tag="ssum", name="ssum")
        nc.scalar.activation(
            et[:], sc_t[:], AF.Exp, bias=nmx[:, 0:1], scale=1.0, accum_out=ssum[:]
        )
        rs = work.tile([IPC, 1], f32, tag="rs", name="rs")
        nc.vector.reciprocal(rs[:], ssum[:])
        attn = work.tile([IPC, N], bf16, tag="attn", name="attn_sb")
        nc.vector.tensor_scalar_mul(attn[:], et[:], rs[:, 0:1])

        # Matmul operands must start at partition 0/32/64, so repack attn rows
        # into 3 partition groups with rows along the free dim.
        GRP = (IPC + 2) // 3  # 43 rows per group
        attn_rows = const.tile([65, GRP * N], bf16, tag="attn_rows", name="attn_rows")
        for g in range(3):
            r0 = g * GRP
            r1 = min(IPC, r0 + GRP)
            nc.sync.dma_start(
                attn_rows[32 * g : 32 * g + 1, 0 : (r1 - r0) * N],
                attn[r0:r1, :],
            )

        # xiT[h, i] = W1i @ z_i.T + b1 ; yjT[h, j] = W1j @ z.T
        xi_ps = ps.tile([H, IPC], f32, tag="mm", name="xi_ps")
        nc.tensor.matmul(xi_ps[:], W1iT_t[:], zTi_t[:], start=True, stop=True)
        xiT_t = const.tile([H, IPC], f32, tag="xiT", name="xiT_sb")
        nc.scalar.activation(xiT_t[:], xi_ps[:], AF.Identity, bias=b1_t[:, 0:1])
        yj_ps = ps.tile([H, N], f32, tag="mm", name="yj_ps")
        nc.tensor.matmul(yj_ps[:], W1jT_t[:], zT_t[:], start=True, stop=True)
        yjT_t = const.tile([H, N], f32, tag="yjT", name="yjT_sb")
        nc.scalar.activation(yjT_t[:], yj_ps[:], AF.Identity, bias=0.0)

        # main loop over this core's 128 i's
        U = const.tile([H, IPC], f32, tag="U", name="U_sb")
        scratch = const.tile([H, N], f32, tag="scratch", name="scratch_sb")
        for i in range(IPC):
            g, r = divmod(i, GRP)
            arep = apool.tile([H, N], f32, tag="arep", name="arep")
            nc.tensor.matmul(
                arep[:],
                ones_t[32 * g : 32 * g + 1, :],
                attn_rows[32 * g : 32 * g + 1, r * N : (r + 1) * N],
                start=True,
                stop=True,
            )
            v = vpool.tile([H, N], f32, tag="v", name="v")
            nc.scalar.activation(
                v[:], yjT_t[:], AF.Tanh, bias=xiT_t[:, i : i + 1], scale=1.0
            )
            nc.vector.scalar_tensor_tensor(
                scratch[:],
                v[:],
                1.0,
                arep[:],
                ALU.mult,
                ALU.mult,
                accum_out=U[:, i : i + 1],
            )

        # epilogue MLP: agg = W2@U + b2 ; t3 = tanh(W3@agg + b3) ; dz = W4@t3 + b4
        c2 = ps.tile([H, IPC], f32, tag="mm", name="c2_ps")
        nc.tensor.matmul(c2[:], W2T_t[:], U[:], start=True, stop=True)
        agg = work.tile([H, IPC], f32, tag="agg", name="agg_sb")
        nc.scalar.activation(agg[:], c2[:], AF.Identity, bias=b2_t[:, 0:1])
        c3 = ps.tile([H, IPC], f32, tag="mm", name="c3_ps")
        nc.tensor.matmul(c3[:], W3T_t[:], agg[:], start=True, stop=True)
        t3 = work.tile([H, IPC], f32, tag="t3", name="t3_sb")
        nc.scalar.activation(t3[:], c3[:], AF.Tanh, bias=b3_t[:, 0:1])
        c4 = ps.tile([H, IPC], f32, tag="mm", name="c4_ps")
        nc.tensor.matmul(c4[:], W4T_t[:], t3[:], start=True, stop=True)
        dzT = work.tile([H, IPC], f32, tag="dzT", name="dzT_sb")
        nc.scalar.activation(dzT[:], c4[:], AF.Identity, bias=b4_t[:, 0:1])
        nc.sync.dma_start(out[:, :], dzT[:])

    nc.finalize()
    return nc


def _get_nc():
    if "nc" not in _CACHE:
        _CACHE["nc"] = _build()
    return _CACHE["nc"]


def kernel(**inputs):
    global LAST_RESULTS
    from concourse.bass_utils import run_bass_kernel_spmd

    z = np.asarray(inputs["z"], dtype=np.float32)
    s_t = np.asarray(inputs["s_t"], dtype=np.float32)
    W1 = np.asarray(inputs["W1"], dtype=np.float32)
    b1 = np.asarray(inputs["b1"], dtype=np.float32)
    W2 = np.asarray(inputs["W2"], dtype=np.float32)
    b2 = np.asarray(inputs["b2"], dtype=np.float32)
    Wq = np.asarray(inputs["Wq"], dtype=np.float32)
    bq = np.asarray(inputs["bq"], dtype=np.float32)
    Wk = np.asarray(inputs["Wk"], dtype=np.float32)
    bk = np.asarray(inputs["bk"], dtype=np.float32)
    W3 = np.asarray(inputs["W3"], dtype=np.float32)
    b3 = np.asarray(inputs["b3"], dtype=np.float32)
    W4 = np.asarray(inputs["W4"], dtype=np.float32)
    b4 = np.asarray(inputs["b4"], dtype=np.float32)

    rt = np.float32(1.0 / np.sqrt(H))
    col = lambda v: np.ascontiguousarray(v.reshape(H, 1), dtype=np.float32)
    tr = lambda m: np.ascontiguousarray(m.T, dtype=np.float32)

    shared = dict(
        ones=np.ones((65, IPC), ml_dtypes.bfloat16),
        WqTs=tr(Wq) * rt,
        bqs=col(bq) * rt,
        WkT=tr(Wk),
        bk=col(bk),
        W1iT=tr(W1[:, :H]),
        b1=col(b1),
        W1jT=tr(W1[:, H:]),
        W2T=tr(W2),
        b2=col(b2),
        W3T=tr(W3),
        b3=col(b3),
        W4T=tr(W4),
        b4=col(b4),
    )

    in_maps = []
    for c in range(NC):
        b, blk = divmod(c, CPB)
        i0 = blk * IPC
        m = np.zeros((IPC, N), np.float32)
        m[np.arange(IPC), i0 + np.arange(IPC)] = np.float32(-1e30)
        in_maps.append(
            dict(
                shared,
                zT=tr(z[b]),
                zTi=tr(z[b, i0 : i0 + IPC]),
                sT=tr(s_t[b]),
                mask=m,
            )
        )

    nc = _get_nc()
    res = run_bass_kernel_spmd(nc, in_maps, core_ids=list(range(NC)))
    LAST_RESULTS = res

    dz = np.empty((B, N, H), dtype=np.float32)
    for c in range(NC):
        b, blk = divmod(c, CPB)
        i0 = blk * IPC
        dz[b, i0 : i0 + IPC, :] = res.results[c]["out"].T
    return dz
